# revision 16
# baseline (speedup 1.0000x reference)
"""Multi-head self-attention (B=2, S=2048, D=2048, H=16) on 8 trn2 cores.

Sharding: tensor-parallel over heads (2 heads/core) for QKV projections and
attention; AllToAll of the normalized per-head context re-shards by token so
the output projection is token-parallel — each core emits a disjoint
512-token slice of the final output and the host gather is a concatenation.

All matmuls run in bf16 with f32 PSUM accumulation. Attention scores are
computed transposed
([k_part, q_free]) so the exp'd scores feed the attn@V matmul directly with
no transposes; softmax denominators come from an all-ones stationary matmul
accumulated in PSUM, and the normalization is a single DVE divide.
V is projected transposed (N=512 matmuls) and flipped to natural layout with
PE transposes. bk is dropped (softmax-invariant). bq is applied via
activation bias; bv/bo via K=1 outer-product matmuls.

Large weight loads go on the gpsimd DMA queue so they don't head-of-line
block the activation stream on the sync queue; weights are pre-laid-out on
the host so every DMA is row-linear.
"""

import numpy as np
from contextlib import ExitStack

import concourse.bacc as bacc
import concourse.bass as bass
import concourse.mybir as mybir
import concourse.tile as tile
from concourse.bass_utils import run_bass_kernel_spmd

B, S, D = 2, 2048, 2048
H, DK = 16, 128
NCORE = 8
HPC = H // NCORE            # heads per core = 2
HD = HPC * DK               # per-core projection width = 256
NT = B * S                  # global tokens = 4096
TG = 512                    # q-group / token-slice width
NTG = NT // TG              # 8
KC = D // 128               # contraction chunks for projections = 16
NKT = S // 128              # k-tiles per (b,h) = 16
TSLICE = NT // NCORE        # tokens per core in output projection = 512
SCALE = float(1.0 / np.sqrt(DK))

F32 = mybir.dt.float32
F32R = mybir.dt.float32r
BF16 = mybir.dt.bfloat16
AF = mybir.ActivationFunctionType
ALU = mybir.AluOpType

_CACHE = {}


def build():
    if "nc" in _CACHE:
        return _CACHE["nc"]

    nc = bacc.Bacc("TRN2", target_bir_lowering=False, debug=False,
                   num_devices=NCORE)

    xT = nc.dram_tensor("xT", [D, NT], BF16, kind="ExternalInput").ap()
    wq = nc.dram_tensor("wq", [128, KC, HD], BF16, kind="ExternalInput").ap()
    wk = nc.dram_tensor("wk", [128, KC, HD], BF16, kind="ExternalInput").ap()
    wv = nc.dram_tensor("wv", [128, KC, HD], BF16, kind="ExternalInput").ap()
    bq = nc.dram_tensor("bq", [HPC, DK], F32, kind="ExternalInput").ap()
    bv = nc.dram_tensor("bv", [1, HD], BF16, kind="ExternalInput").ap()
    wo = nc.dram_tensor("wo", [D, D], BF16, kind="ExternalInput").ap()
    bo = nc.dram_tensor("bo", [1, D], BF16, kind="ExternalInput").ap()
    masks = nc.dram_tensor("masks", [128, 4 * TG], BF16,
                           kind="ExternalInput").ap()
    ones_in = nc.dram_tensor("ones_in", [128, TG], BF16,
                             kind="ExternalInput").ap()
    ident_in = nc.dram_tensor("ident_in", [128, 128], BF16,
                              kind="ExternalInput").ap()
    out = nc.dram_tensor("out", [TSLICE, D], F32, kind="ExternalOutput").ap()

    with tile.TileContext(nc) as tc, ExitStack() as ctx:
        const = ctx.enter_context(tc.tile_pool(name="const", bufs=1))
        dram = ctx.enter_context(tc.tile_pool(name="dram", bufs=1,
                                              space="DRAM"))

        ones = const.tile([128, TG], BF16)
        nc.gpsimd.dma_start(ones[:], ones_in)
        ident = const.tile([128, 128], BF16)
        nc.gpsimd.dma_start(ident[:], ident_in)
        masks_sb = const.tile([128, 4 * TG], BF16)
        nc.gpsimd.dma_start(masks_sb[:], masks)
        bq_sb = const.tile([128, HPC], F32)
        nc.gpsimd.dma_start(bq_sb[:], bq.rearrange("h p -> p h"))
        bv_sb = const.tile([1, HD], BF16)
        nc.gpsimd.dma_start(bv_sb[:], bv)
        bo_sb = const.tile([1, D], BF16)
        nc.gpsimd.dma_start(bo_sb[:], bo)

        # DRAM scratch
        qT_d = dram.tile([HPC * DK, NT], BF16)      # Q^T  per head
        kT_d = dram.tile([HPC * DK, NT], BF16)      # K^T  per head
        v_d = dram.tile([HPC * NT, DK], BF16)       # V natural per head
        a2a_in = [dram.tile([NCORE * DK, TG], BF16, name=f"a2a_in{h}",
                            tag=f"a2a_in{h}") for h in range(HPC)]
        a2a_out = [dram.tile([NCORE * DK, TG], BF16, name=f"a2a_out{h}",
                             tag=f"a2a_out{h}") for h in range(HPC)]

        # ---------------- Phase 1: QKV projections ----------------
        with (
            tc.tile_pool(name="w1", bufs=1) as w1,
            tc.tile_pool(name="xp", bufs=18) as xp,
            tc.tile_pool(name="ev", bufs=3) as ev,
            tc.tile_pool(name="ps1", bufs=2, space="PSUM") as ps1,
        ):
            wq_sb = w1.tile([128, KC, HD], BF16)
            nc.gpsimd.dma_start(wq_sb[:], wq)
            wk_sb = w1.tile([128, KC, HD], BF16)
            nc.gpsimd.dma_start(wk_sb[:], wk)
            wv_sb = w1.tile([128, KC, HD], BF16)
            nc.gpsimd.dma_start(wv_sb[:], wv)

            for tg in range(NTG):
                t0 = tg * TG
                xts = []
                for c in range(KC):
                    xt = xp.tile([128, TG], BF16, tag="xt")
                    nc.sync.dma_start(xt[:], xT[c * 128:(c + 1) * 128,
                                                t0:t0 + TG])
                    xts.append(xt)

                # Q^T / K^T: psum[hd, t] += W^T_chunk.T @ xT_chunk
                for wsb, dest, bias in ((wq_sb, qT_d, True),
                                        (wk_sb, kT_d, False)):
                    for h in range(HPC):
                        ps = ps1.tile([128, TG], F32, tag="ps", bufs=3)
                        for c in range(KC):
                            nc.tensor.matmul(
                                ps[:],
                                lhsT=wsb[:, c, h * DK:(h + 1) * DK],
                                rhs=xts[c][:],
                                start=(c == 0), stop=(c == KC - 1))
                        e = ev.tile([128, TG], BF16, tag="ev")
                        if bias:
                            nc.scalar.activation(e[:], ps[:],
                                                 AF.Identity,
                                                 bias=bq_sb[:, h:h + 1])
                        else:
                            nc.scalar.copy(e[:], ps[:])
                        nc.sync.dma_start(
                            dest[h * DK:(h + 1) * DK, t0:t0 + TG], e[:])

                # V^T (N=512 matmuls), then PE-transpose to natural layout
                for h in range(HPC):
                    pv = ps1.tile([128, TG], F32, tag="pv")
                    for c in range(KC):
                        nc.tensor.matmul(
                            pv[:], lhsT=wv_sb[:, c, h * DK:(h + 1) * DK],
                            rhs=xts[c][:], start=(c == 0), stop=False)
                    nc.tensor.matmul(
                        pv[:], lhsT=bv_sb[0:1, h * DK:(h + 1) * DK],
                        rhs=ones[0:1, :], start=False, stop=True)
                    vt = ev.tile([128, TG], BF16, tag="vt")
                    nc.scalar.copy(vt[:], pv[:])
                    for ts in range(TG // 128):
                        ptr = ps1.tile([128, 128], BF16, tag="tr")
                        nc.tensor.transpose(
                            ptr[:], vt[:, ts * 128:(ts + 1) * 128], ident[:])
                        vs = ev.tile([128, 128], BF16, tag="vs")
                        nc.vector.tensor_copy(vs[:], ptr[:])
                        tt = t0 + ts * 128
                        nc.sync.dma_start(
                            v_d[h * NT + tt:h * NT + tt + 128, :], vs[:])

        # ---------------- Phase 2: attention per (h, b) ----------------
        ph2 = ExitStack()
        ld2 = ph2.enter_context(tc.tile_pool(name="ld2", bufs=2))
        pp = ph2.enter_context(tc.tile_pool(name="pp", bufs=3))
        ps2 = ph2.enter_context(tc.tile_pool(name="ps2", bufs=2,
                                             space="PSUM"))
        for h in range(HPC):
            for b in range(B):
                qT_sb = ld2.tile([128, S], BF16, tag="qT")
                nc.sync.dma_start(
                    qT_sb[:], qT_d[h * DK:(h + 1) * DK, b * S:(b + 1) * S])
                kT_sb = ld2.tile([128, S], BF16, tag="kT")
                nc.sync.dma_start(
                    kT_sb[:], kT_d[h * DK:(h + 1) * DK, b * S:(b + 1) * S])
                v_sb = ld2.tile([128, NKT, DK], BF16, tag="v")
                nc.sync.dma_start(
                    v_sb[:],
                    v_d[h * NT + b * S:h * NT + (b + 1) * S, :]
                    .rearrange("(nt p) d -> p nt d", p=128))

                for qg in range(S // TG):
                    nk = (TG // 128) * (qg + 1)   # causal: lower k-tiles only
                    ps_ctx = ps2.tile([128, TG], F32, tag="ctx")
                    ps_sum = ps2.tile([128, TG], F32, tag="sum")
                    for kt in range(nk):
                        ps_s = ps2.tile([128, TG], F32, tag="s", bufs=3)
                        nc.tensor.matmul(
                            ps_s[:],
                            lhsT=kT_sb[:, kt * 128:(kt + 1) * 128],
                            rhs=qT_sb[:, qg * TG:(qg + 1) * TG],
                            start=True, stop=True)
                        p_sb = pp.tile([128, TG], BF16, tag="p", bufs=4)
                        nc.scalar.activation(p_sb[:], ps_s[:], AF.Exp,
                                             scale=SCALE)
                        diag = kt - (nk - 4)
                        if diag >= 0:
                            nc.vector.tensor_mul(
                                p_sb[:], p_sb[:],
                                masks_sb[:, diag * TG:(diag + 1) * TG])
                        nc.tensor.matmul(
                            ps_ctx[:], lhsT=v_sb[:, kt, :], rhs=p_sb[:],
                            start=(kt == 0), stop=(kt == nk - 1))
                        nc.tensor.matmul(
                            ps_sum[:], lhsT=ones[:, 0:128], rhs=p_sb[:],
                            start=(kt == 0), stop=(kt == nk - 1))
                    r_sb = pp.tile([128, TG], F32, tag="r")
                    nc.vector.reciprocal_approx_fast(r_sb[:], ps_sum[:])
                    cn_sb = pp.tile([128, TG], BF16, tag="cn")
                    nc.vector.tensor_mul(cn_sb[:], ps_ctx[:], r_sb[:])
                    j = b * (S // TG) + qg
                    nc.sync.dma_start(
                        a2a_in[h][j * DK:(j + 1) * DK, :], cn_sb[:])

            nc.gpsimd.collective_compute(
                "AllToAll", ALU.bypass,
                replica_groups=[list(range(NCORE))],
                ins=[a2a_in[h][:].opt()],
                outs=[a2a_out[h][:].opt()])

        ph2.close()

        # ---------------- Phase 3: output projection ----------------
        with (
            tc.tile_pool(name="cx", bufs=1) as cx,
            tc.tile_pool(name="wop", bufs=20) as wop,
            tc.tile_pool(name="ev3", bufs=3) as ev3,
            tc.tile_pool(name="ps3", bufs=4, space="PSUM") as ps3,
        ):
            ctx_sb = cx.tile([128, H, TG], BF16)
            for hh in range(HPC):
                for src in range(NCORE):
                    nc.sync.dma_start(
                        ctx_sb[:, 2 * src + hh, :],
                        a2a_out[hh][src * DK:(src + 1) * DK, :])

            # accumulate h=0 chunks first so the last AllToAll overlaps
            gorder = [2 * s for s in range(NCORE)] + \
                     [2 * s + 1 for s in range(NCORE)]
            for ic in range(D // TG):
                i0 = ic * TG
                wts = {}
                for g in gorder:
                    wt = wop.tile([128, TG], BF16, tag="wo")
                    nc.gpsimd.dma_start(
                        wt[:], wo[g * 128:(g + 1) * 128, i0:i0 + TG])
                    wts[g] = wt
                for ts in range(TSLICE // 128):
                    po = ps3.tile([128, TG], F32, tag="po")
                    for n, g in enumerate(gorder):
                        nc.tensor.matmul(
                            po[:],
                            lhsT=ctx_sb[:, g, ts * 128:(ts + 1) * 128],
                            rhs=wts[g][:],
                            start=(n == 0), stop=False)
                    nc.tensor.matmul(po[:], lhsT=ones[0:1, 0:128],
                                     rhs=bo_sb[0:1, i0:i0 + TG],
                                     start=False, stop=True)
                    oe = ev3.tile([128, TG], F32, tag="oe")
                    nc.scalar.copy(oe[:], po[:])
                    nc.sync.dma_start(
                        out[ts * 128:(ts + 1) * 128, i0:i0 + TG], oe[:])

    nc.compile()
    _CACHE["nc"] = nc
    return nc


def make_in_maps(x, Wq, bq, Wk, Wv, bv, Wo, bo):
    import ml_dtypes
    bf = ml_dtypes.bfloat16
    xT = np.ascontiguousarray(x.reshape(NT, D).T).astype(bf)
    woT = np.ascontiguousarray(Wo.T).astype(bf)
    m = np.zeros((4, 128, TG), np.float32)
    kk = np.arange(128)[:, None]
    qq = np.arange(TG)[None, :]
    for i in range(4):
        m[i] = (kk + 128 * i <= qq).astype(np.float32)
    m = np.ascontiguousarray(m.transpose(1, 0, 2).reshape(128, 4 * TG))

    def wlay(W, lo, hi):
        # [D, HD] -> [p=128, c=KC, HD] contiguous so the DMA is row-linear
        wt = W[lo:hi, :].T.reshape(KC, 128, HD)
        return np.ascontiguousarray(wt.transpose(1, 0, 2)).astype(bf)

    in_maps = []
    for c in range(NCORE):
        lo, hi = c * HD, (c + 1) * HD
        in_maps.append({
            "xT": xT,
            "wq": wlay(Wq, lo, hi),
            "wk": wlay(Wk, lo, hi),
            "wv": wlay(Wv, lo, hi),
            "bq": np.ascontiguousarray(bq[lo:hi].reshape(HPC, DK)).astype(np.float32),
            "bv": np.ascontiguousarray(bv[lo:hi].reshape(1, HD)).astype(bf),
            "wo": woT,
            "bo": np.ascontiguousarray(bo.reshape(1, D)).astype(bf),
            "masks": m.astype(bf),
            "ones_in": np.ones((128, TG), bf),
            "ident_in": np.eye(128, dtype=np.float32).astype(bf),
        })
    return in_maps


def run_sharded(inputs, trace=False, **kwargs):
    nc = build()
    in_maps = make_in_maps(
        np.asarray(inputs["x"]), np.asarray(inputs["Wq"]),
        np.asarray(inputs["bq"]), np.asarray(inputs["Wk"]),
        np.asarray(inputs["Wv"]), np.asarray(inputs["bv"]),
        np.asarray(inputs["Wo"]), np.asarray(inputs["bo"]))
    res = run_bass_kernel_spmd(nc, in_maps, core_ids=list(range(NCORE)),
                               trace=trace, **kwargs)
    slices = [res.results[c]["out"] for c in range(NCORE)]
    full = np.concatenate(slices, axis=0).reshape(B, S, D).astype(np.float32)
    return full, res


def kernel(**inputs) -> np.ndarray:
    full, _ = run_sharded(inputs, trace=False)
    return full


# revision 17
# speedup vs baseline: 1.0418x; 1.0418x over previous
"""Multi-head self-attention (B=2, S=2048, D=2048, H=16) on 8 trn2 cores.

Sharding: tensor-parallel over heads (2 heads/core) for QKV projections and
attention; AllToAll of the normalized per-head context re-shards by token so
the output projection is token-parallel — each core emits a disjoint
512-token slice of the final output and the host gather is a concatenation.

All matmuls run in bf16 with f32 PSUM accumulation. Attention scores are
computed transposed
([k_part, q_free]) so the exp'd scores feed the attn@V matmul directly with
no transposes; softmax denominators come from an all-ones stationary matmul
accumulated in PSUM, and the normalization is a single DVE divide.
V is projected transposed (N=512 matmuls) and flipped to natural layout with
PE transposes. bk is dropped (softmax-invariant). bq is applied via
activation bias; bv/bo via K=1 outer-product matmuls.

Large weight loads go on the gpsimd DMA queue so they don't head-of-line
block the activation stream on the sync queue; weights are pre-laid-out on
the host so every DMA is row-linear.
"""

import numpy as np
from contextlib import ExitStack

import concourse.bacc as bacc
import concourse.bass as bass
import concourse.mybir as mybir
import concourse.tile as tile
from concourse.bass_utils import run_bass_kernel_spmd

B, S, D = 2, 2048, 2048
H, DK = 16, 128
NCORE = 8
HPC = H // NCORE            # heads per core = 2
HD = HPC * DK               # per-core projection width = 256
NT = B * S                  # global tokens = 4096
TG = 512                    # q-group / token-slice width
NTG = NT // TG              # 8
KC = D // 128               # contraction chunks for projections = 16
NKT = S // 128              # k-tiles per (b,h) = 16
TSLICE = NT // NCORE        # tokens per core in output projection = 512
SCALE = float(1.0 / np.sqrt(DK))

F32 = mybir.dt.float32
F32R = mybir.dt.float32r
BF16 = mybir.dt.bfloat16
AF = mybir.ActivationFunctionType
ALU = mybir.AluOpType

_CACHE = {}


def build():
    if "nc" in _CACHE:
        return _CACHE["nc"]

    nc = bacc.Bacc("TRN2", target_bir_lowering=False, debug=False,
                   num_devices=NCORE)

    xT = nc.dram_tensor("xT", [D, NT], BF16, kind="ExternalInput").ap()
    wq = nc.dram_tensor("wq", [128, KC, HD], BF16, kind="ExternalInput").ap()
    wk = nc.dram_tensor("wk", [128, KC, HD], BF16, kind="ExternalInput").ap()
    wv = nc.dram_tensor("wv", [128, KC, HD], BF16, kind="ExternalInput").ap()
    bq = nc.dram_tensor("bq", [HPC, DK], F32, kind="ExternalInput").ap()
    bv = nc.dram_tensor("bv", [1, HD], BF16, kind="ExternalInput").ap()
    wo = nc.dram_tensor("wo", [D, D], BF16, kind="ExternalInput").ap()
    bo = nc.dram_tensor("bo", [1, D], BF16, kind="ExternalInput").ap()
    masks = nc.dram_tensor("masks", [128, 4 * TG], BF16,
                           kind="ExternalInput").ap()
    ones_in = nc.dram_tensor("ones_in", [128, TG], BF16,
                             kind="ExternalInput").ap()
    ident_in = nc.dram_tensor("ident_in", [128, 128], BF16,
                              kind="ExternalInput").ap()
    out = nc.dram_tensor("out", [TSLICE, D], F32, kind="ExternalOutput").ap()

    with tile.TileContext(nc) as tc, ExitStack() as ctx:
        const = ctx.enter_context(tc.tile_pool(name="const", bufs=1))
        dram = ctx.enter_context(tc.tile_pool(name="dram", bufs=1,
                                              space="DRAM"))

        ones = const.tile([128, TG], BF16)
        nc.gpsimd.dma_start(ones[:], ones_in)
        ident = const.tile([128, 128], BF16)
        nc.gpsimd.dma_start(ident[:], ident_in)
        masks_sb = const.tile([128, 4 * TG], BF16)
        nc.gpsimd.dma_start(masks_sb[:], masks)
        bq_sb = const.tile([128, HPC], F32)
        nc.gpsimd.dma_start(bq_sb[:], bq.rearrange("h p -> p h"))
        bv_sb = const.tile([1, HD], BF16)
        nc.gpsimd.dma_start(bv_sb[:], bv)
        bo_sb = const.tile([1, D], BF16)
        nc.gpsimd.dma_start(bo_sb[:], bo)

        # DRAM scratch
        qT_d = dram.tile([HPC * DK, NT], BF16)      # Q^T  per head
        kT_d = dram.tile([HPC * DK, NT], BF16)      # K^T  per head
        v_d = dram.tile([HPC * NT, DK], BF16)       # V natural per head
        a2a_in = [dram.tile([NCORE * DK, TG], BF16, name=f"a2a_in{h}",
                            tag=f"a2a_in{h}") for h in range(HPC)]
        a2a_out = [dram.tile([NCORE * DK, TG], BF16, name=f"a2a_out{h}",
                             tag=f"a2a_out{h}") for h in range(HPC)]

        # ---------------- Phase 1: QKV projections ----------------
        with (
            tc.tile_pool(name="w1", bufs=1) as w1,
            tc.tile_pool(name="xp", bufs=18) as xp,
            tc.tile_pool(name="ev", bufs=3) as ev,
            tc.tile_pool(name="ps1", bufs=2, space="PSUM") as ps1,
        ):
            wq_sb = w1.tile([128, KC, HD], BF16)
            nc.gpsimd.dma_start(wq_sb[:], wq)
            wk_sb = w1.tile([128, KC, HD], BF16)
            nc.gpsimd.dma_start(wk_sb[:], wk)
            wv_sb = w1.tile([128, KC, HD], BF16)
            nc.gpsimd.dma_start(wv_sb[:], wv)

            for tg in range(NTG):
                t0 = tg * TG
                xts = []
                for c in range(KC):
                    xt = xp.tile([128, TG], BF16, tag="xt")
                    nc.sync.dma_start(xt[:], xT[c * 128:(c + 1) * 128,
                                                t0:t0 + TG])
                    xts.append(xt)

                # Q^T / K^T: psum[hd, t] += W^T_chunk.T @ xT_chunk
                for wsb, dest, bias in ((wq_sb, qT_d, True),
                                        (wk_sb, kT_d, False)):
                    for h in range(HPC):
                        ps = ps1.tile([128, TG], F32, tag="ps", bufs=3)
                        for c in range(KC):
                            nc.tensor.matmul(
                                ps[:],
                                lhsT=wsb[:, c, h * DK:(h + 1) * DK],
                                rhs=xts[c][:],
                                start=(c == 0), stop=(c == KC - 1))
                        e = ev.tile([128, TG], BF16, tag="ev")
                        if bias:
                            nc.scalar.activation(e[:], ps[:],
                                                 AF.Identity,
                                                 bias=bq_sb[:, h:h + 1])
                        else:
                            nc.scalar.copy(e[:], ps[:])
                        nc.sync.dma_start(
                            dest[h * DK:(h + 1) * DK, t0:t0 + TG], e[:])

                # V^T (N=512 matmuls), then PE-transpose to natural layout
                for h in range(HPC):
                    pv = ps1.tile([128, TG], F32, tag="pv")
                    for c in range(KC):
                        nc.tensor.matmul(
                            pv[:], lhsT=wv_sb[:, c, h * DK:(h + 1) * DK],
                            rhs=xts[c][:], start=(c == 0), stop=False)
                    nc.tensor.matmul(
                        pv[:], lhsT=bv_sb[0:1, h * DK:(h + 1) * DK],
                        rhs=ones[0:1, :], start=False, stop=True)
                    vt = ev.tile([128, TG], BF16, tag="vt")
                    nc.scalar.copy(vt[:], pv[:])
                    for ts in range(TG // 128):
                        ptr = ps1.tile([128, 128], BF16, tag="tr")
                        nc.tensor.transpose(
                            ptr[:], vt[:, ts * 128:(ts + 1) * 128], ident[:])
                        vs = ev.tile([128, 128], BF16, tag="vs")
                        nc.vector.tensor_copy(vs[:], ptr[:])
                        tt = t0 + ts * 128
                        nc.sync.dma_start(
                            v_d[h * NT + tt:h * NT + tt + 128, :], vs[:])

        # ---------------- Phase 2: attention per (h, b) ----------------
        ph2 = ExitStack()
        ld2 = ph2.enter_context(tc.tile_pool(name="ld2", bufs=2))
        pp = ph2.enter_context(tc.tile_pool(name="pp", bufs=3))
        ps2 = ph2.enter_context(tc.tile_pool(name="ps2", bufs=2,
                                             space="PSUM"))
        for h in range(HPC):
            for b in range(B):
                qT_sb = ld2.tile([128, S], BF16, tag="qT")
                nc.sync.dma_start(
                    qT_sb[:], qT_d[h * DK:(h + 1) * DK, b * S:(b + 1) * S])
                kT_sb = ld2.tile([128, S], BF16, tag="kT")
                nc.sync.dma_start(
                    kT_sb[:], kT_d[h * DK:(h + 1) * DK, b * S:(b + 1) * S])
                v_sb = ld2.tile([128, NKT, DK], BF16, tag="v")
                nc.sync.dma_start(
                    v_sb[:],
                    v_d[h * NT + b * S:h * NT + (b + 1) * S, :]
                    .rearrange("(nt p) d -> p nt d", p=128))

                for qg in range(S // TG):
                    nk = (TG // 128) * (qg + 1)   # causal: lower k-tiles only
                    ps_ctx = ps2.tile([128, TG], F32, tag="ctx")
                    ps_sum = ps2.tile([128, TG], F32, tag="sum")
                    pts = []
                    for kt in range(nk):
                        ps_s = ps2.tile([128, TG], F32, tag="s", bufs=3)
                        nc.tensor.matmul(
                            ps_s[:],
                            lhsT=kT_sb[:, kt * 128:(kt + 1) * 128],
                            rhs=qT_sb[:, qg * TG:(qg + 1) * TG],
                            start=True, stop=True)
                        p_sb = pp.tile([128, TG], BF16, tag="p", bufs=18)
                        nc.scalar.activation(p_sb[:], ps_s[:], AF.Exp,
                                             scale=SCALE)
                        diag = kt - (nk - 4)
                        if diag >= 0:
                            nc.vector.tensor_mul(
                                p_sb[:], p_sb[:],
                                masks_sb[:, diag * TG:(diag + 1) * TG])
                        nc.tensor.matmul(
                            ps_ctx[:], lhsT=v_sb[:, kt, :], rhs=p_sb[:],
                            start=(kt == 0), stop=(kt == nk - 1))
                        pts.append(p_sb)
                    # denominators: batched afterwards so these matmuls are
                    # free-floating PE filler, not part of the exp chain
                    for kt in range(nk):
                        nc.tensor.matmul(
                            ps_sum[:], lhsT=ones[:, 0:128], rhs=pts[kt][:],
                            start=(kt == 0), stop=(kt == nk - 1))
                    r_sb = pp.tile([128, TG], F32, tag="r")
                    nc.vector.reciprocal_approx_fast(r_sb[:], ps_sum[:])
                    cn_sb = pp.tile([128, TG], BF16, tag="cn")
                    nc.vector.tensor_mul(cn_sb[:], ps_ctx[:], r_sb[:])
                    j = b * (S // TG) + qg
                    nc.sync.dma_start(
                        a2a_in[h][j * DK:(j + 1) * DK, :], cn_sb[:])

            nc.gpsimd.collective_compute(
                "AllToAll", ALU.bypass,
                replica_groups=[list(range(NCORE))],
                ins=[a2a_in[h][:].opt()],
                outs=[a2a_out[h][:].opt()])

        ph2.close()

        # ---------------- Phase 3: output projection ----------------
        with (
            tc.tile_pool(name="cx", bufs=1) as cx,
            tc.tile_pool(name="wop", bufs=20) as wop,
            tc.tile_pool(name="ev3", bufs=3) as ev3,
            tc.tile_pool(name="ps3", bufs=4, space="PSUM") as ps3,
        ):
            ctx_sb = cx.tile([128, H, TG], BF16)
            for hh in range(HPC):
                for src in range(NCORE):
                    nc.sync.dma_start(
                        ctx_sb[:, 2 * src + hh, :],
                        a2a_out[hh][src * DK:(src + 1) * DK, :])

            # accumulate h=0 chunks first so the last AllToAll overlaps
            gorder = [2 * s for s in range(NCORE)] + \
                     [2 * s + 1 for s in range(NCORE)]
            for ic in range(D // TG):
                i0 = ic * TG
                wts = {}
                for g in gorder:
                    wt = wop.tile([128, TG], BF16, tag="wo")
                    nc.gpsimd.dma_start(
                        wt[:], wo[g * 128:(g + 1) * 128, i0:i0 + TG])
                    wts[g] = wt
                for ts in range(TSLICE // 128):
                    po = ps3.tile([128, TG], F32, tag="po")
                    for n, g in enumerate(gorder):
                        nc.tensor.matmul(
                            po[:],
                            lhsT=ctx_sb[:, g, ts * 128:(ts + 1) * 128],
                            rhs=wts[g][:],
                            start=(n == 0), stop=False)
                    nc.tensor.matmul(po[:], lhsT=ones[0:1, 0:128],
                                     rhs=bo_sb[0:1, i0:i0 + TG],
                                     start=False, stop=True)
                    oe = ev3.tile([128, TG], F32, tag="oe")
                    nc.scalar.copy(oe[:], po[:])
                    nc.sync.dma_start(
                        out[ts * 128:(ts + 1) * 128, i0:i0 + TG], oe[:])

    nc.compile()
    _CACHE["nc"] = nc
    return nc


def make_in_maps(x, Wq, bq, Wk, Wv, bv, Wo, bo):
    import ml_dtypes
    bf = ml_dtypes.bfloat16
    xT = np.ascontiguousarray(x.reshape(NT, D).T).astype(bf)
    woT = np.ascontiguousarray(Wo.T).astype(bf)
    m = np.zeros((4, 128, TG), np.float32)
    kk = np.arange(128)[:, None]
    qq = np.arange(TG)[None, :]
    for i in range(4):
        m[i] = (kk + 128 * i <= qq).astype(np.float32)
    m = np.ascontiguousarray(m.transpose(1, 0, 2).reshape(128, 4 * TG))

    def wlay(W, lo, hi):
        # [D, HD] -> [p=128, c=KC, HD] contiguous so the DMA is row-linear
        wt = W[lo:hi, :].T.reshape(KC, 128, HD)
        return np.ascontiguousarray(wt.transpose(1, 0, 2)).astype(bf)

    in_maps = []
    for c in range(NCORE):
        lo, hi = c * HD, (c + 1) * HD
        in_maps.append({
            "xT": xT,
            "wq": wlay(Wq, lo, hi),
            "wk": wlay(Wk, lo, hi),
            "wv": wlay(Wv, lo, hi),
            "bq": np.ascontiguousarray(bq[lo:hi].reshape(HPC, DK)).astype(np.float32),
            "bv": np.ascontiguousarray(bv[lo:hi].reshape(1, HD)).astype(bf),
            "wo": woT,
            "bo": np.ascontiguousarray(bo.reshape(1, D)).astype(bf),
            "masks": m.astype(bf),
            "ones_in": np.ones((128, TG), bf),
            "ident_in": np.eye(128, dtype=np.float32).astype(bf),
        })
    return in_maps


def run_sharded(inputs, trace=False, **kwargs):
    nc = build()
    in_maps = make_in_maps(
        np.asarray(inputs["x"]), np.asarray(inputs["Wq"]),
        np.asarray(inputs["bq"]), np.asarray(inputs["Wk"]),
        np.asarray(inputs["Wv"]), np.asarray(inputs["bv"]),
        np.asarray(inputs["Wo"]), np.asarray(inputs["bo"]))
    res = run_bass_kernel_spmd(nc, in_maps, core_ids=list(range(NCORE)),
                               trace=trace, **kwargs)
    slices = [res.results[c]["out"] for c in range(NCORE)]
    full = np.concatenate(slices, axis=0).reshape(B, S, D).astype(np.float32)
    return full, res


def kernel(**inputs) -> np.ndarray:
    full, _ = run_sharded(inputs, trace=False)
    return full


# revision 18
# speedup vs baseline: 1.0518x; 1.0096x over previous
"""Multi-head self-attention (B=2, S=2048, D=2048, H=16) on 8 trn2 cores.

Sharding: tensor-parallel over heads (2 heads/core) for QKV projections and
attention; AllToAll of the normalized per-head context re-shards by token so
the output projection is token-parallel — each core emits a disjoint
512-token slice of the final output and the host gather is a concatenation.

All matmuls run in bf16 with f32 PSUM accumulation. Attention scores are
computed transposed
([k_part, q_free]) so the exp'd scores feed the attn@V matmul directly with
no transposes; softmax denominators come from an all-ones stationary matmul
accumulated in PSUM, and the normalization is a single DVE divide.
V is projected transposed (N=512 matmuls) and flipped to natural layout with
PE transposes. bk is dropped (softmax-invariant). bq is applied via
activation bias; bv/bo via K=1 outer-product matmuls.

Large weight loads go on the gpsimd DMA queue so they don't head-of-line
block the activation stream on the sync queue; weights are pre-laid-out on
the host so every DMA is row-linear.
"""

import numpy as np
from contextlib import ExitStack

import concourse.bacc as bacc
import concourse.bass as bass
import concourse.mybir as mybir
import concourse.tile as tile
from concourse.bass_utils import run_bass_kernel_spmd

B, S, D = 2, 2048, 2048
H, DK = 16, 128
NCORE = 8
HPC = H // NCORE            # heads per core = 2
HD = HPC * DK               # per-core projection width = 256
NT = B * S                  # global tokens = 4096
TG = 512                    # q-group / token-slice width
NTG = NT // TG              # 8
KC = D // 128               # contraction chunks for projections = 16
NKT = S // 128              # k-tiles per (b,h) = 16
TSLICE = NT // NCORE        # tokens per core in output projection = 512
SCALE = float(1.0 / np.sqrt(DK))

F32 = mybir.dt.float32
F32R = mybir.dt.float32r
BF16 = mybir.dt.bfloat16
AF = mybir.ActivationFunctionType
ALU = mybir.AluOpType

_CACHE = {}


def build():
    if "nc" in _CACHE:
        return _CACHE["nc"]

    nc = bacc.Bacc("TRN2", target_bir_lowering=False, debug=False,
                   num_devices=NCORE)

    xT = nc.dram_tensor("xT", [D, NT], BF16, kind="ExternalInput").ap()
    wq = nc.dram_tensor("wq", [128, KC, HD], BF16, kind="ExternalInput").ap()
    wk = nc.dram_tensor("wk", [128, KC, HD], BF16, kind="ExternalInput").ap()
    wv = nc.dram_tensor("wv", [128, KC, HD], BF16, kind="ExternalInput").ap()
    bq = nc.dram_tensor("bq", [HPC, DK], F32, kind="ExternalInput").ap()
    bv = nc.dram_tensor("bv", [1, HD], BF16, kind="ExternalInput").ap()
    wo = nc.dram_tensor("wo", [D, D], BF16, kind="ExternalInput").ap()
    bo = nc.dram_tensor("bo", [1, D], BF16, kind="ExternalInput").ap()
    masks = nc.dram_tensor("masks", [128, 4 * TG], BF16,
                           kind="ExternalInput").ap()
    ones_in = nc.dram_tensor("ones_in", [128, TG], BF16,
                             kind="ExternalInput").ap()
    ident_in = nc.dram_tensor("ident_in", [128, 128], BF16,
                              kind="ExternalInput").ap()
    out = nc.dram_tensor("out", [TSLICE, D], F32, kind="ExternalOutput").ap()

    with tile.TileContext(nc) as tc, ExitStack() as ctx:
        const = ctx.enter_context(tc.tile_pool(name="const", bufs=1))
        dram = ctx.enter_context(tc.tile_pool(name="dram", bufs=1,
                                              space="DRAM"))
        cx = ctx.enter_context(tc.tile_pool(name="cx", bufs=1))
        wop = ctx.enter_context(tc.tile_pool(name="wop", bufs=20))

        ones = const.tile([128, TG], BF16)
        nc.gpsimd.dma_start(ones[:], ones_in)
        ident = const.tile([128, 128], BF16)
        nc.gpsimd.dma_start(ident[:], ident_in)
        masks_sb = const.tile([128, 4 * TG], BF16)
        nc.gpsimd.dma_start(masks_sb[:], masks)
        bq_sb = const.tile([128, HPC], F32)
        nc.gpsimd.dma_start(bq_sb[:], bq.rearrange("h p -> p h"))
        bv_sb = const.tile([1, HD], BF16)
        nc.gpsimd.dma_start(bv_sb[:], bv)
        bo_sb = const.tile([1, D], BF16)
        nc.gpsimd.dma_start(bo_sb[:], bo)

        # DRAM scratch
        qT_d = dram.tile([HPC * DK, NT], BF16)      # Q^T  per head
        kT_d = dram.tile([HPC * DK, NT], BF16)      # K^T  per head
        v_d = dram.tile([HPC * NT, DK], BF16)       # V natural per head
        a2a_in = [dram.tile([NCORE * DK, TG], BF16, name=f"a2a_in{h}",
                            tag=f"a2a_in{h}") for h in range(HPC)]
        a2a_out = [dram.tile([NCORE * DK, TG], BF16, name=f"a2a_out{h}",
                             tag=f"a2a_out{h}") for h in range(HPC)]

        # ---------------- Phase 1: QKV projections ----------------
        with (
            tc.tile_pool(name="w1", bufs=1) as w1,
            tc.tile_pool(name="xp", bufs=18) as xp,
            tc.tile_pool(name="ev", bufs=3) as ev,
            tc.tile_pool(name="ps1", bufs=2, space="PSUM") as ps1,
        ):
            wq_sb = w1.tile([128, KC, HD], BF16)
            nc.gpsimd.dma_start(wq_sb[:], wq)
            wk_sb = w1.tile([128, KC, HD], BF16)
            nc.gpsimd.dma_start(wk_sb[:], wk)
            wv_sb = w1.tile([128, KC, HD], BF16)
            nc.gpsimd.dma_start(wv_sb[:], wv)

            for tg in range(NTG):
                t0 = tg * TG
                xts = []
                for c in range(KC):
                    xt = xp.tile([128, TG], BF16, tag="xt")
                    nc.sync.dma_start(xt[:], xT[c * 128:(c + 1) * 128,
                                                t0:t0 + TG])
                    xts.append(xt)

                # Q^T / K^T: psum[hd, t] += W^T_chunk.T @ xT_chunk
                for wsb, dest, bias in ((wq_sb, qT_d, True),
                                        (wk_sb, kT_d, False)):
                    for h in range(HPC):
                        ps = ps1.tile([128, TG], F32, tag="ps", bufs=3)
                        for c in range(KC):
                            nc.tensor.matmul(
                                ps[:],
                                lhsT=wsb[:, c, h * DK:(h + 1) * DK],
                                rhs=xts[c][:],
                                start=(c == 0), stop=(c == KC - 1))
                        e = ev.tile([128, TG], BF16, tag="ev")
                        if bias:
                            nc.scalar.activation(e[:], ps[:],
                                                 AF.Identity,
                                                 bias=bq_sb[:, h:h + 1])
                        else:
                            nc.scalar.copy(e[:], ps[:])
                        nc.sync.dma_start(
                            dest[h * DK:(h + 1) * DK, t0:t0 + TG], e[:])

                # V^T (N=512 matmuls), then PE-transpose to natural layout
                for h in range(HPC):
                    pv = ps1.tile([128, TG], F32, tag="pv")
                    for c in range(KC):
                        nc.tensor.matmul(
                            pv[:], lhsT=wv_sb[:, c, h * DK:(h + 1) * DK],
                            rhs=xts[c][:], start=(c == 0), stop=False)
                    nc.tensor.matmul(
                        pv[:], lhsT=bv_sb[0:1, h * DK:(h + 1) * DK],
                        rhs=ones[0:1, :], start=False, stop=True)
                    vt = ev.tile([128, TG], BF16, tag="vt")
                    nc.scalar.copy(vt[:], pv[:])
                    for ts in range(TG // 128):
                        ptr = ps1.tile([128, 128], BF16, tag="tr")
                        nc.tensor.transpose(
                            ptr[:], vt[:, ts * 128:(ts + 1) * 128], ident[:])
                        vs = ev.tile([128, 128], BF16, tag="vs")
                        nc.vector.tensor_copy(vs[:], ptr[:])
                        tt = t0 + ts * 128
                        nc.sync.dma_start(
                            v_d[h * NT + tt:h * NT + tt + 128, :], vs[:])

        # ---------------- Phase 2: attention per (h, b) ----------------
        ph2 = ExitStack()
        ld2 = ph2.enter_context(tc.tile_pool(name="ld2", bufs=2))
        pp = ph2.enter_context(tc.tile_pool(name="pp", bufs=3))
        ps2 = ph2.enter_context(tc.tile_pool(name="ps2", bufs=2,
                                             space="PSUM"))
        for h in range(HPC):
            for b in range(B):
                qT_sb = ld2.tile([128, S], BF16, tag="qT")
                nc.sync.dma_start(
                    qT_sb[:], qT_d[h * DK:(h + 1) * DK, b * S:(b + 1) * S])
                kT_sb = ld2.tile([128, S], BF16, tag="kT")
                nc.sync.dma_start(
                    kT_sb[:], kT_d[h * DK:(h + 1) * DK, b * S:(b + 1) * S])
                v_sb = ld2.tile([128, NKT, DK], BF16, tag="v")
                nc.sync.dma_start(
                    v_sb[:],
                    v_d[h * NT + b * S:h * NT + (b + 1) * S, :]
                    .rearrange("(nt p) d -> p nt d", p=128))

                for qg in range(S // TG):
                    nk = (TG // 128) * (qg + 1)   # causal: lower k-tiles only
                    ps_ctx = ps2.tile([128, TG], F32, tag="ctx")
                    ps_sum = ps2.tile([128, TG], F32, tag="sum")
                    pts = []
                    for kt in range(nk):
                        ps_s = ps2.tile([128, TG], F32, tag="s", bufs=3)
                        nc.tensor.matmul(
                            ps_s[:],
                            lhsT=kT_sb[:, kt * 128:(kt + 1) * 128],
                            rhs=qT_sb[:, qg * TG:(qg + 1) * TG],
                            start=True, stop=True)
                        p_sb = pp.tile([128, TG], BF16, tag="p", bufs=18)
                        nc.scalar.activation(p_sb[:], ps_s[:], AF.Exp,
                                             scale=SCALE)
                        diag = kt - (nk - 4)
                        if diag >= 0:
                            nc.vector.tensor_mul(
                                p_sb[:], p_sb[:],
                                masks_sb[:, diag * TG:(diag + 1) * TG])
                        nc.tensor.matmul(
                            ps_ctx[:], lhsT=v_sb[:, kt, :], rhs=p_sb[:],
                            start=(kt == 0), stop=(kt == nk - 1))
                        pts.append(p_sb)
                    # denominators: batched afterwards so these matmuls are
                    # free-floating PE filler, not part of the exp chain
                    for kt in range(nk):
                        nc.tensor.matmul(
                            ps_sum[:], lhsT=ones[:, 0:128], rhs=pts[kt][:],
                            start=(kt == 0), stop=(kt == nk - 1))
                    r_sb = pp.tile([128, TG], F32, tag="r")
                    nc.vector.reciprocal_approx_fast(r_sb[:], ps_sum[:])
                    cn_sb = pp.tile([128, TG], BF16, tag="cn")
                    nc.vector.tensor_mul(cn_sb[:], ps_ctx[:], r_sb[:])
                    j = b * (S // TG) + qg
                    nc.sync.dma_start(
                        a2a_in[h][j * DK:(j + 1) * DK, :], cn_sb[:])

            nc.gpsimd.collective_compute(
                "AllToAll", ALU.bypass,
                replica_groups=[list(range(NCORE))],
                ins=[a2a_in[h][:].opt()],
                outs=[a2a_out[h][:].opt()])

        ph2.close()

        # ---------------- Phase 3: output projection ----------------
        with (
            tc.tile_pool(name="ev3", bufs=3) as ev3,
            tc.tile_pool(name="ps3", bufs=4, space="PSUM") as ps3,
        ):
            ctx_sb = cx.tile([128, H, TG], BF16)
            for hh in range(HPC):
                for src in range(NCORE):
                    nc.sync.dma_start(
                        ctx_sb[:, 2 * src + hh, :],
                        a2a_out[hh][src * DK:(src + 1) * DK, :])

            # accumulate h=0 chunks first so the last AllToAll overlaps
            gorder = [2 * s for s in range(NCORE)] + \
                     [2 * s + 1 for s in range(NCORE)]
            for ic in range(D // TG):
                i0 = ic * TG
                wts = {}
                for g in gorder:
                    wt = wop.tile([128, TG], BF16, tag="wo")
                    nc.gpsimd.dma_start(
                        wt[:], wo[g * 128:(g + 1) * 128, i0:i0 + TG])
                    wts[g] = wt
                for ts in range(TSLICE // 128):
                    po = ps3.tile([128, TG], F32, tag="po")
                    for n, g in enumerate(gorder):
                        nc.tensor.matmul(
                            po[:],
                            lhsT=ctx_sb[:, g, ts * 128:(ts + 1) * 128],
                            rhs=wts[g][:],
                            start=(n == 0), stop=False)
                    nc.tensor.matmul(po[:], lhsT=ones[0:1, 0:128],
                                     rhs=bo_sb[0:1, i0:i0 + TG],
                                     start=False, stop=True)
                    oe = ev3.tile([128, TG], F32, tag="oe")
                    nc.scalar.copy(oe[:], po[:])
                    nc.sync.dma_start(
                        out[ts * 128:(ts + 1) * 128, i0:i0 + TG], oe[:])

    nc.compile()
    _CACHE["nc"] = nc
    return nc


def make_in_maps(x, Wq, bq, Wk, Wv, bv, Wo, bo):
    import ml_dtypes
    bf = ml_dtypes.bfloat16
    xT = np.ascontiguousarray(x.reshape(NT, D).T).astype(bf)
    woT = np.ascontiguousarray(Wo.T).astype(bf)
    m = np.zeros((4, 128, TG), np.float32)
    kk = np.arange(128)[:, None]
    qq = np.arange(TG)[None, :]
    for i in range(4):
        m[i] = (kk + 128 * i <= qq).astype(np.float32)
    m = np.ascontiguousarray(m.transpose(1, 0, 2).reshape(128, 4 * TG))

    def wlay(W, lo, hi):
        # [D, HD] -> [p=128, c=KC, HD] contiguous so the DMA is row-linear
        wt = W[lo:hi, :].T.reshape(KC, 128, HD)
        return np.ascontiguousarray(wt.transpose(1, 0, 2)).astype(bf)

    in_maps = []
    for c in range(NCORE):
        lo, hi = c * HD, (c + 1) * HD
        in_maps.append({
            "xT": xT,
            "wq": wlay(Wq, lo, hi),
            "wk": wlay(Wk, lo, hi),
            "wv": wlay(Wv, lo, hi),
            "bq": np.ascontiguousarray(bq[lo:hi].reshape(HPC, DK)).astype(np.float32),
            "bv": np.ascontiguousarray(bv[lo:hi].reshape(1, HD)).astype(bf),
            "wo": woT,
            "bo": np.ascontiguousarray(bo.reshape(1, D)).astype(bf),
            "masks": m.astype(bf),
            "ones_in": np.ones((128, TG), bf),
            "ident_in": np.eye(128, dtype=np.float32).astype(bf),
        })
    return in_maps


def run_sharded(inputs, trace=False, **kwargs):
    nc = build()
    in_maps = make_in_maps(
        np.asarray(inputs["x"]), np.asarray(inputs["Wq"]),
        np.asarray(inputs["bq"]), np.asarray(inputs["Wk"]),
        np.asarray(inputs["Wv"]), np.asarray(inputs["bv"]),
        np.asarray(inputs["Wo"]), np.asarray(inputs["bo"]))
    res = run_bass_kernel_spmd(nc, in_maps, core_ids=list(range(NCORE)),
                               trace=trace, **kwargs)
    slices = [res.results[c]["out"] for c in range(NCORE)]
    full = np.concatenate(slices, axis=0).reshape(B, S, D).astype(np.float32)
    return full, res


def kernel(**inputs) -> np.ndarray:
    full, _ = run_sharded(inputs, trace=False)
    return full
